# revision 3
# baseline (speedup 1.0000x reference)
"""Trainium2 Bass kernel for the ODEFunc problem (time-conditioned MLP + exact
divergence of the Jacobian), data-parallel over 8 NeuronCores.

Math (per sample row z):
    x1 = z @ W1[:64] + (b1 + t*W1[64])          # t-column folded into bias
    h1 = silu(x1);  s1 = silu'(x1)
    x2 = h1 @ W2 + b2
    h2 = silu(x2);  s2 = silu'(x2)
    dz = h2 @ W3 + b3
    div = rowsum((s1 @ C) * s2),  C = W2 * (W1[:64].T @ W3.T)
    dlogp_dt = -div
silu'(x) is computed on-device from h = silu(x) and T = tanh(x/2) via
    silu'(x) = (1 + T + h*(1-T)) / 2
so the ACT engine only ever needs the {silu, tanh} LUT set (one table load).
The derivative assembly is a single fused custom-DVE op.

Device layout: feature-major activations [128 feat, batch] per 128-feature
chunk; z is transposed on entry with PE transposes; the last layer uses the
h2-chunks as the stationary matmul operand to produce batch-major output
directly; divergence column-sum is a matmul against a constant -1 vector
(which also folds the dlogp negation).
"""
import sys

if '/opt/trn_rl_repo' not in sys.path:
    sys.path.insert(0, '/opt/trn_rl_repo')

import numpy as np

B, D, H = 16384, 64, 256
N_CORES = 8
BC = B // N_CORES          # 2048 rows per core
G = 1024                   # rows per group (a-tile free dim)
NG = BC // G               # groups per core
NSUB = G // 128            # 128-row subtiles per group

_compiled = {}


def _register_custom_dve_op():
    """Register SILU_BWD_FUSED_ANT: out = (1 + T + h*(1-T)) * 0.5 with
    in0=h, in1=T.  Appended to the concourse custom-DVE registry (rows
    1..0x1f are free; we take the next one after the production ops)."""
    import concourse.dve_ops as dve_ops
    from concourse.dve_spec import Spec, Src0, Src1, C2, One, lower, _has_src1
    from concourse.dve_uop import DveOpSpec

    if any(op.name == "SILU_BWD_FUSED_ANT" for op in dve_ops.OPS):
        return next(op for op in dve_ops.OPS if op.name == "SILU_BWD_FUSED_ANT")

    spec = Spec(
        body=((One + Src1) + Src0 * (One - Src1)) * C2,
        reference=lambda in0, in1, s0, s1, imm2: (
            (1.0 + in1.astype(np.float32)) + in0 * (1.0 - in1)
        ) * imm2,
    )
    op = dve_ops.DveOp(
        "SILU_BWD_FUSED_ANT",
        spec,
        subdim=False,
        uops_sha={"v3": "1dc4e106a000efc1", "v4": "9590f733b321b289"},
    )
    dve_ops.OPS.append(op)
    dve_ops.CUSTOM_DVE_SPECS[op.name] = op.spec
    dve_ops._SUB_OPCODE_FOR_NAME[op.name] = (
        dve_ops._CUSTOM_DVE_ROW_BASE + len(dve_ops.OPS) - 1
    )
    return op


def _build():
    import concourse.bacc as bacc
    import concourse.tile as tile
    import concourse.mybir as mybir
    from concourse.masks import make_identity

    silu_bwd = _register_custom_dve_op()

    dt = mybir.dt.float32
    A = mybir.ActivationFunctionType

    nc = bacc.Bacc("TRN2", target_bir_lowering=False, debug=False,
                   num_devices=N_CORES)

    z_d = nc.dram_tensor("z", [BC, D], dt, kind="ExternalInput").ap()
    w1_d = nc.dram_tensor("w1", [D, H], dt, kind="ExternalInput").ap()
    w2_d = nc.dram_tensor("w2", [H, H], dt, kind="ExternalInput").ap()
    w3_d = nc.dram_tensor("w3", [H, D], dt, kind="ExternalInput").ap()
    cmat_d = nc.dram_tensor("cmat", [H, H], dt, kind="ExternalInput").ap()
    bias_d = nc.dram_tensor("biases", [128, 8], dt, kind="ExternalInput").ap()
    b3r_d = nc.dram_tensor("b3rep", [128, 512], dt, kind="ExternalInput").ap()

    dz_d = nc.dram_tensor("dz", [BC, D], dt, kind="ExternalOutput").ap()
    dlp_d = nc.dram_tensor("dlp", [BC], dt, kind="ExternalOutput").ap()

    from contextlib import ExitStack

    with tile.TileContext(nc) as tc, ExitStack() as ctx:
        consts = ctx.enter_context(tc.tile_pool(name="consts", bufs=1))
        zin_p = ctx.enter_context(tc.tile_pool(name="zin", bufs=2))
        ztsb_p = ctx.enter_context(tc.tile_pool(name="ztsb", bufs=2))
        act_p = ctx.enter_context(tc.tile_pool(name="acts", bufs=2))
        out_p = ctx.enter_context(tc.tile_pool(name="outs", bufs=2))
        dlp_p = ctx.enter_context(tc.tile_pool(name="dlps", bufs=2))
        ps_a = ctx.enter_context(tc.tile_pool(name="ps_a", bufs=2, space="PSUM"))
        ps_zt = ctx.enter_context(tc.tile_pool(name="ps_zt", bufs=2, space="PSUM"))
        ps_out = ctx.enter_context(tc.tile_pool(name="ps_out", bufs=1, space="PSUM"))
        ps_div = ctx.enter_context(tc.tile_pool(name="ps_div", bufs=1, space="PSUM"))

        # --- constants ---
        w1sb = consts.tile([D, H], dt)
        nc.sync.dma_start(w1sb, w1_d)
        w2sb = consts.tile([128, 2, H], dt)
        nc.sync.dma_start(w2sb, w2_d.rearrange("(k p) n -> p k n", p=128))
        csb = consts.tile([128, 2, H], dt)
        nc.sync.dma_start(csb, cmat_d.rearrange("(k p) n -> p k n", p=128))
        w3sb = consts.tile([128, 2, D], dt)
        nc.sync.dma_start(w3sb, w3_d.rearrange("(k p) n -> p k n", p=128))
        biassb = consts.tile([128, 8], dt)
        nc.sync.dma_start(biassb, bias_d)
        b3sb = consts.tile([128, 512], dt)
        nc.sync.dma_start(b3sb, b3r_d)
        neg1 = consts.tile([128, 1], dt)
        nc.vector.memset(neg1, -1.0)
        ident = consts.tile([128, 128], dt)
        make_identity(nc, ident[:])

        z3 = z_d.rearrange("(g s p) d -> g s p d", g=NG, p=128)
        dz3 = dz_d.rearrange("(g s p) d -> g s p d", g=NG, p=128)

        for g in range(NG):
            # --- z load + transpose (feature-major z.T in SBUF) ---
            zin = zin_p.tile([128, NSUB, D], dt)
            nc.sync.dma_start(zin, z3[g].rearrange("s p d -> p s d"))
            ztsb = ztsb_p.tile([D, G], dt)
            for q in range(G // 512):
                ztps = ps_zt.tile([D, 512], dt)
                for s in range(4):
                    nc.tensor.transpose(
                        ztps[:, s * 128:(s + 1) * 128],
                        zin[:, q * 4 + s, :], ident[:])
                nc.vector.tensor_copy(ztsb[:, q * 512:(q + 1) * 512], ztps)

            # --- layer 1 ---
            h1, t1, s1 = {}, {}, {}
            for m in range(2):
                a1 = ps_a.tile([128, G], dt, tag="a")
                for q in range(G // 512):
                    nc.tensor.matmul(
                        a1[:, q * 512:(q + 1) * 512],
                        lhsT=w1sb[:, m * 128:(m + 1) * 128],
                        rhs=ztsb[:, q * 512:(q + 1) * 512],
                        start=True, stop=True)
                h1[m] = act_p.tile([128, G], dt, tag=f"h1_{m}", name=f"h1_{m}_{g}")
                nc.scalar.activation(h1[m], a1, A.Silu,
                                     bias=biassb[:, 0 + m:1 + m])
                t1[m] = act_p.tile([128, G], dt, tag=f"t1_{m}", name=f"t1_{m}_{g}")
                nc.scalar.activation(t1[m], a1, A.Tanh,
                                     bias=biassb[:, 2 + m:3 + m], scale=0.5)
                s1[m] = act_p.tile([128, G], dt, tag=f"s1_{m}", name=f"s1_{m}_{g}")
                nc.vector._custom_dve(silu_bwd, out=s1[m][:], in0=h1[m][:],
                                      in1=t1[m][:], imm2=0.5)

            # --- layer 2 ---
            h2, t2, s2 = {}, {}, {}
            for m in range(2):
                a2 = ps_a.tile([128, G], dt, tag="a")
                for q in range(G // 512):
                    for k in range(2):
                        nc.tensor.matmul(
                            a2[:, q * 512:(q + 1) * 512],
                            lhsT=w2sb[:, k, m * 128:(m + 1) * 128],
                            rhs=h1[k][:, q * 512:(q + 1) * 512],
                            start=(k == 0), stop=(k == 1))
                h2[m] = act_p.tile([128, G], dt, tag=f"h2_{m}", name=f"h2_{m}_{g}")
                nc.scalar.activation(h2[m], a2, A.Silu,
                                     bias=biassb[:, 4 + m:5 + m])
                t2[m] = act_p.tile([128, G], dt, tag=f"t2_{m}", name=f"t2_{m}_{g}")
                nc.scalar.activation(t2[m], a2, A.Tanh,
                                     bias=biassb[:, 6 + m:7 + m], scale=0.5)
                s2[m] = act_p.tile([128, G], dt, tag=f"s2_{m}", name=f"s2_{m}_{g}")
                nc.vector._custom_dve(silu_bwd, out=s2[m][:], in0=h2[m][:],
                                      in1=t2[m][:], imm2=0.5)

            # --- layer 3: batch-major output (h2 chunks as stationary) ---
            outps = ps_out.tile([128, NSUB * D], dt)
            for s in range(NSUB):
                for k in range(2):
                    nc.tensor.matmul(
                        outps[:, s * D:(s + 1) * D],
                        lhsT=h2[k][:, s * 128:(s + 1) * 128],
                        rhs=w3sb[:, k, :],
                        start=(k == 0), stop=(k == 1))
            outsb = out_p.tile([128, NSUB * D], dt)
            nc.vector.tensor_add(outsb, outps, b3sb[:, :NSUB * D])
            nc.sync.dma_start(dz3[g].rearrange("s p d -> p s d"),
                              outsb[:].rearrange("p (s d) -> p s d", d=D))

            # --- divergence: v = C^T-chunks @ s1, w = v*s2, dlp = -colsum ---
            w = {}
            for m in range(2):
                vps = ps_a.tile([128, G], dt, tag="a")
                for q in range(G // 512):
                    for k in range(2):
                        nc.tensor.matmul(
                            vps[:, q * 512:(q + 1) * 512],
                            lhsT=csb[:, k, m * 128:(m + 1) * 128],
                            rhs=s1[k][:, q * 512:(q + 1) * 512],
                            start=(k == 0), stop=(k == 1))
                w[m] = act_p.tile([128, G], dt, tag=f"w_{m}", name=f"w_{m}_{g}")
                nc.vector.tensor_mul(w[m], vps, s2[m])

            for q in range(G // 512):
                divps = ps_div.tile([1, 512], dt)
                for k in range(2):
                    nc.tensor.matmul(
                        divps,
                        lhsT=neg1,
                        rhs=w[k][:, q * 512:(q + 1) * 512],
                        start=(k == 0), stop=(k == 1))
                dlpsb = dlp_p.tile([1, 512], dt)
                nc.vector.tensor_copy(dlpsb, divps)
                nc.sync.dma_start(
                    dlp_d[g * G + q * 512:g * G + (q + 1) * 512]
                    .rearrange("(a b) -> a b", a=1),
                    dlpsb)

    nc.compile()
    return nc


def _get_compiled():
    if "nc" not in _compiled:
        _compiled["nc"] = _build()
    return _compiled["nc"]


def kernel(t, z, logp, W1, b1, W2, b2, W3, b3):
    from concourse.bass_utils import run_bass_kernel_spmd

    t = np.asarray(t, np.float32)
    z = np.ascontiguousarray(np.asarray(z, np.float32))
    W1 = np.asarray(W1, np.float32)
    b1 = np.asarray(b1, np.float32)
    W2 = np.ascontiguousarray(np.asarray(W2, np.float32))
    b2 = np.asarray(b2, np.float32)
    W3 = np.ascontiguousarray(np.asarray(W3, np.float32))
    b3 = np.asarray(b3, np.float32)

    # host-side prep of the tiny weight-derived constants
    b1e = b1 + t[0] * W1[D]                          # t column folded into bias
    cmat = np.ascontiguousarray(W2 * (W1[:D].T @ W3.T))
    biases = np.stack([b1e[:128], b1e[128:],
                       0.5 * b1e[:128], 0.5 * b1e[128:],
                       b2[:128], b2[128:],
                       0.5 * b2[:128], 0.5 * b2[128:]], axis=1)
    biases = np.ascontiguousarray(biases, np.float32)  # [128, 8]
    b3rep = np.ascontiguousarray(np.tile(b3, (128, 8)), np.float32)  # [128,512]
    w1c = np.ascontiguousarray(W1[:D])

    nc = _get_compiled()
    shared = {"w1": w1c, "w2": W2, "w3": W3, "cmat": cmat,
              "biases": biases, "b3rep": b3rep}
    in_maps = [dict(shared, z=np.ascontiguousarray(z[c * BC:(c + 1) * BC]))
               for c in range(N_CORES)]
    res = run_bass_kernel_spmd(nc, in_maps, core_ids=list(range(N_CORES)))

    dz = np.concatenate([res.results[c]["dz"] for c in range(N_CORES)], axis=0)
    dlp = np.concatenate([res.results[c]["dlp"] for c in range(N_CORES)], axis=0)
    return dz, dlp


# revision 9
# speedup vs baseline: 1.5792x; 1.5792x over previous
"""Trainium2 Bass kernel for the ODEFunc problem (time-conditioned MLP + exact
divergence of the Jacobian), data-parallel over 8 NeuronCores.

Math (per sample row z):
    x1 = z @ W1[:64] + (b1 + t*W1[64])          # t-column folded into bias
    h1 = silu(x1);  s1 = silu'(x1)
    x2 = h1 @ W2 + b2
    h2 = silu(x2);  s2 = silu'(x2)
    dz = h2 @ W3 + b3
    div = rowsum((s1 @ C) * s2),  C = W2 * (W1[:64].T @ W3.T)
    dlogp_dt = -div
silu'(x) is computed on-device from h = silu(x) and T = tanh(x/2) via
    silu'(x) = (1 + T + h*(1-T)) / 2
so the ACT engine only ever needs the {silu, tanh} LUT set (one table load).
The derivative assembly is a single fused custom-DVE op.

Device layout: feature-major activations [128 feat, batch] per 128-feature
chunk; z is transposed on entry with PE transposes; the last layer uses the
h2-chunks as the stationary matmul operand to produce batch-major output
directly; divergence column-sum is a matmul against a constant -1 vector
(which also folds the dlogp negation).
"""
import sys

if '/opt/trn_rl_repo' not in sys.path:
    sys.path.insert(0, '/opt/trn_rl_repo')

import numpy as np

B, D, H = 16384, 64, 256
N_CORES = 8
BC = B // N_CORES          # 2048 rows per core
G = 1024                   # rows per group (a-tile free dim)
NG = BC // G               # groups per core
NSUB = G // 128            # 128-row subtiles per group

_compiled = {}


def _register_custom_dve_op():
    """Register SILU_BWD_FUSED_ANT: out = (1 + T + h*(1-T)) * 0.5 with
    in0=h, in1=T.  Appended to the concourse custom-DVE registry (rows
    1..0x1f are free; we take the next one after the production ops)."""
    import concourse.dve_ops as dve_ops
    from concourse.dve_spec import Spec, Src0, Src1, C2, One, lower, _has_src1
    from concourse.dve_uop import DveOpSpec

    if any(op.name == "SILU_BWD_FUSED_ANT" for op in dve_ops.OPS):
        return next(op for op in dve_ops.OPS if op.name == "SILU_BWD_FUSED_ANT")

    spec = Spec(
        body=((One + Src1) + Src0 * (One - Src1)) * C2,
        reference=lambda in0, in1, s0, s1, imm2: (
            (1.0 + in1.astype(np.float32)) + in0 * (1.0 - in1)
        ) * imm2,
    )
    op = dve_ops.DveOp(
        "SILU_BWD_FUSED_ANT",
        spec,
        subdim=False,
        uops_sha={"v3": "1dc4e106a000efc1", "v4": "9590f733b321b289"},
    )
    dve_ops.OPS.append(op)
    dve_ops.CUSTOM_DVE_SPECS[op.name] = op.spec
    dve_ops._SUB_OPCODE_FOR_NAME[op.name] = (
        dve_ops._CUSTOM_DVE_ROW_BASE + len(dve_ops.OPS) - 1
    )
    return op


def _build():
    import concourse.bacc as bacc
    import concourse.tile as tile
    import concourse.mybir as mybir

    silu_bwd = _register_custom_dve_op()

    dt = mybir.dt.float32
    dtr = mybir.dt.float32r
    A = mybir.ActivationFunctionType

    nc = bacc.Bacc("TRN2", target_bir_lowering=False, debug=False,
                   num_devices=N_CORES)

    z_d = nc.dram_tensor("z", [BC, D], dtr, kind="ExternalInput").ap()
    w1_d = nc.dram_tensor("w1", [D, H], dtr, kind="ExternalInput").ap()
    w2_d = nc.dram_tensor("w2", [H, H], dtr, kind="ExternalInput").ap()
    w3_d = nc.dram_tensor("w3", [H, D], dtr, kind="ExternalInput").ap()
    cmat_d = nc.dram_tensor("cmat", [H, H], dtr, kind="ExternalInput").ap()
    bias_d = nc.dram_tensor("biases", [128, 8], dt, kind="ExternalInput").ap()
    ident_d = nc.dram_tensor("identm", [128, 128], dtr, kind="ExternalInput").ap()
    neg1_d = nc.dram_tensor("negones", [128, 1], dtr, kind="ExternalInput").ap()
    b3r_d = nc.dram_tensor("b3rep", [128, 512], dt, kind="ExternalInput").ap()

    dz_d = nc.dram_tensor("dz", [BC, D], dt, kind="ExternalOutput").ap()
    dlp_d = nc.dram_tensor("dlp", [BC], dt, kind="ExternalOutput").ap()

    from contextlib import ExitStack

    with tile.TileContext(nc) as tc, ExitStack() as ctx:
        consts = ctx.enter_context(tc.tile_pool(name="consts", bufs=1))
        zin_p = ctx.enter_context(tc.tile_pool(name="zin", bufs=2))
        ztsb_p = ctx.enter_context(tc.tile_pool(name="ztsb", bufs=2))
        act_p = ctx.enter_context(tc.tile_pool(name="acts", bufs=2))
        out_p = ctx.enter_context(tc.tile_pool(name="outs", bufs=2))
        dlp_p = ctx.enter_context(tc.tile_pool(name="dlps", bufs=2))
        ps_a = ctx.enter_context(tc.tile_pool(name="ps_a", bufs=2, space="PSUM"))
        ps_zt = ctx.enter_context(tc.tile_pool(name="ps_zt", bufs=2, space="PSUM"))
        ps_out = ctx.enter_context(tc.tile_pool(name="ps_out", bufs=1, space="PSUM"))
        ps_div = ctx.enter_context(tc.tile_pool(name="ps_div", bufs=1, space="PSUM"))

        # --- constants ---
        w1sb = consts.tile([D, H], dtr)
        nc.sync.dma_start(w1sb, w1_d)
        w2sb = consts.tile([128, 2, H], dtr)
        nc.sync.dma_start(w2sb, w2_d.rearrange("(k p) n -> p k n", p=128))
        csb = consts.tile([128, 2, H], dtr)
        nc.sync.dma_start(csb, cmat_d.rearrange("(k p) n -> p k n", p=128))
        w3sb = consts.tile([128, 2, D], dtr)
        nc.sync.dma_start(w3sb, w3_d.rearrange("(k p) n -> p k n", p=128))
        biassb = consts.tile([128, 8], dt)
        nc.sync.dma_start(biassb, bias_d)
        b3sb = consts.tile([128, 512], dt)
        nc.sync.dma_start(b3sb, b3r_d)
        neg1 = consts.tile([128, 1], dtr)
        nc.sync.dma_start(neg1, neg1_d)
        ident = consts.tile([128, 128], dtr)
        nc.sync.dma_start(ident, ident_d)

        z3 = z_d.rearrange("(g s p) d -> g s p d", g=NG, p=128)
        dz3 = dz_d.rearrange("(g s p) d -> g s p d", g=NG, p=128)

        for g in range(NG):
            # --- z load + transpose (feature-major z.T in SBUF) ---
            zin = zin_p.tile([128, NSUB, D], dtr)
            nc.sync.dma_start(zin, z3[g].rearrange("s p d -> p s d"))
            ztsb = ztsb_p.tile([D, G], dtr)
            for q in range(G // 512):
                ztps = ps_zt.tile([D, 512], dt)
                for s in range(4):
                    nc.tensor.transpose(
                        ztps[:, s * 128:(s + 1) * 128].bitcast(dtr),
                        zin[:, q * 4 + s, :], ident[:])
                nc.vector.tensor_copy(ztsb[:, q * 512:(q + 1) * 512], ztps)

            # --- layer 1 ---
            h1, t1, s1 = {}, {}, {}
            for m in range(2):
                a1 = ps_a.tile([128, G], dt, tag="a")
                for q in range(G // 512):
                    nc.tensor.matmul(
                        a1[:, q * 512:(q + 1) * 512],
                        lhsT=w1sb[:, m * 128:(m + 1) * 128],
                        rhs=ztsb[:, q * 512:(q + 1) * 512],
                        start=True, stop=True)
                h1[m] = act_p.tile([128, G], dtr, tag=f"h1_{m}", name=f"h1_{m}_{g}")
                nc.scalar.activation(h1[m], a1, A.Silu,
                                     bias=biassb[:, 0 + m:1 + m])
                t1[m] = act_p.tile([128, G], dt, tag=f"t1_{m}", name=f"t1_{m}_{g}")
                nc.scalar.activation(t1[m], a1, A.Tanh,
                                     bias=biassb[:, 2 + m:3 + m], scale=0.5)
                s1[m] = act_p.tile([128, G], dtr, tag=f"s1_{m}", name=f"s1_{m}_{g}")
                nc.vector._custom_dve(silu_bwd, out=s1[m][:], in0=h1[m][:],
                                      in1=t1[m][:], imm2=0.5)

            # --- layer 2 ---
            h2, t2, s2 = {}, {}, {}
            for m in range(2):
                a2 = ps_a.tile([128, G], dt, tag="a")
                for q in range(G // 512):
                    for k in range(2):
                        nc.tensor.matmul(
                            a2[:, q * 512:(q + 1) * 512],
                            lhsT=w2sb[:, k, m * 128:(m + 1) * 128],
                            rhs=h1[k][:, q * 512:(q + 1) * 512],
                            start=(k == 0), stop=(k == 1))
                h2[m] = act_p.tile([128, G], dtr, tag=f"h2_{m}", name=f"h2_{m}_{g}")
                nc.scalar.activation(h2[m], a2, A.Silu,
                                     bias=biassb[:, 4 + m:5 + m])
                t2[m] = act_p.tile([128, G], dt, tag=f"t2_{m}", name=f"t2_{m}_{g}")
                nc.scalar.activation(t2[m], a2, A.Tanh,
                                     bias=biassb[:, 6 + m:7 + m], scale=0.5)
                s2[m] = act_p.tile([128, G], dt, tag=f"s2_{m}", name=f"s2_{m}_{g}")
                nc.vector._custom_dve(silu_bwd, out=s2[m][:], in0=h2[m][:],
                                      in1=t2[m][:], imm2=0.5)

            # --- layer 3: batch-major output (h2 chunks as stationary) ---
            outps = ps_out.tile([128, NSUB * D], dt)
            for s in range(NSUB):
                for k in range(2):
                    nc.tensor.matmul(
                        outps[:, s * D:(s + 1) * D],
                        lhsT=h2[k][:, s * 128:(s + 1) * 128],
                        rhs=w3sb[:, k, :],
                        start=(k == 0), stop=(k == 1))
            outsb = out_p.tile([128, NSUB * D], dt)
            nc.vector.tensor_add(outsb, outps, b3sb[:, :NSUB * D])
            nc.sync.dma_start(dz3[g].rearrange("s p d -> p s d"),
                              outsb[:].rearrange("p (s d) -> p s d", d=D))

            # --- divergence: v = C^T-chunks @ s1, w = v*s2, dlp = -colsum ---
            w = {}
            for m in range(2):
                vps = ps_a.tile([128, G], dt, tag="a")
                for q in range(G // 512):
                    for k in range(2):
                        nc.tensor.matmul(
                            vps[:, q * 512:(q + 1) * 512],
                            lhsT=csb[:, k, m * 128:(m + 1) * 128],
                            rhs=s1[k][:, q * 512:(q + 1) * 512],
                            start=(k == 0), stop=(k == 1))
                w[m] = act_p.tile([128, G], dtr, tag=f"w_{m}", name=f"w_{m}_{g}")
                nc.vector.tensor_mul(w[m], vps, s2[m])

            for q in range(G // 512):
                divps = ps_div.tile([1, 512], dt)
                for k in range(2):
                    nc.tensor.matmul(
                        divps,
                        lhsT=neg1[:],
                        rhs=w[k][:, q * 512:(q + 1) * 512],
                        start=(k == 0), stop=(k == 1))
                dlpsb = dlp_p.tile([1, 512], dt)
                nc.vector.tensor_copy(dlpsb, divps)
                nc.sync.dma_start(
                    dlp_d[g * G + q * 512:g * G + (q + 1) * 512]
                    .rearrange("(a b) -> a b", a=1),
                    dlpsb)

    nc.compile()
    return nc


def _get_compiled():
    if "nc" not in _compiled:
        _compiled["nc"] = _build()
    return _compiled["nc"]


def make_in_maps(t, z, logp, W1, b1, W2, b2, W3, b3):
    t = np.asarray(t, np.float32)
    z = np.ascontiguousarray(np.asarray(z, np.float32))
    W1 = np.asarray(W1, np.float32)
    b1 = np.asarray(b1, np.float32)
    W2 = np.ascontiguousarray(np.asarray(W2, np.float32))
    b2 = np.asarray(b2, np.float32)
    W3 = np.ascontiguousarray(np.asarray(W3, np.float32))
    b3 = np.asarray(b3, np.float32)

    # host-side prep of the tiny weight-derived constants
    b1e = b1 + t[0] * W1[D]                          # t column folded into bias
    cmat = np.ascontiguousarray(W2 * (W1[:D].T @ W3.T))
    biases = np.stack([b1e[:128], b1e[128:],
                       0.5 * b1e[:128], 0.5 * b1e[128:],
                       b2[:128], b2[128:],
                       0.5 * b2[:128], 0.5 * b2[128:]], axis=1)
    biases = np.ascontiguousarray(biases, np.float32)  # [128, 8]
    b3rep = np.ascontiguousarray(np.tile(b3, (128, 8)), np.float32)  # [128,512]
    w1c = np.ascontiguousarray(W1[:D])
    shared = {"w1": w1c, "w2": W2, "w3": W3, "cmat": cmat,
              "biases": biases, "b3rep": b3rep,
              "identm": np.eye(128, dtype=np.float32),
              "negones": np.full((128, 1), -1.0, np.float32)}
    return [dict(shared, z=np.ascontiguousarray(z[c * BC:(c + 1) * BC]))
            for c in range(N_CORES)]


def kernel(t, z, logp, W1, b1, W2, b2, W3, b3):
    from concourse.bass_utils import run_bass_kernel_spmd
    in_maps = make_in_maps(t, z, logp, W1, b1, W2, b2, W3, b3)
    nc = _get_compiled()
    res = run_bass_kernel_spmd(nc, in_maps, core_ids=list(range(N_CORES)))

    dz = np.concatenate([res.results[c]["dz"] for c in range(N_CORES)], axis=0)
    dlp = np.concatenate([res.results[c]["dlp"] for c in range(N_CORES)], axis=0)
    return dz, dlp
